# revision 13
# baseline (speedup 1.0000x reference)
"""MultiHeadGraphAttention TRN2 kernel, v3.

Data-parallel over (batch, query-half); core c: batch c//2, query rows
(c%2)*1024..+1024.

v3 vs v2 (279us):
 - Projections (Q/K/V/O) run fp8e4m3 DoubleRow matmuls: 2 contraction
   subtiles of 128 per instruction -> half the PE column count.  DR
   stationary width must be a multiple of 32.
 - exp+mask runs on one of three per-group paths to balance ACT/DVE/GPSIMD:
     a: ACT exp->bf16 (bias -4), DVE bf16 mask-mult (2x mode), bf16 AV
     b: DVE scalar_tensor_tensor Schraudolph: int16(s*a16 + Bm) bitcast
        bf16 = exp(s-4)*mask fused in ONE 1x op; bf16 AV
     c: ACT exp->fp8 (bias -4), GPSIMD fp8 mask-mult (proxy ucode lib
        has TensorTensor + PartitionBroadcast together -> no lib thrash),
        fp8 DR AV with 96-wide V slots (64 V + ones col + 31 zero cols;
        zero cols are free: PE time scales with moving columns only)
   The -4 shift keeps exp in fp8e4m3 range; softmax-invariant.
 - oT is fp8 -> O-projection also DR.
 - normalize: denominator row copies PSUM->SBUF moved to ACT (Copy lives
   in the exp table set -> no ACT table reload), reciprocal via the PE
   transpose dance as v2.
 - LayerNorm tail unchanged from v2 (batched Sqrt waves).
"""

import os
import sys

import numpy as np

try:
    import concourse  # noqa: F401
except ImportError:
    sys.path.insert(0, "/opt/trn_rl_repo")

import ml_dtypes

B, N, M, D, H, HD = 4, 2048, 2048, 512, 8, 64
NS = 1024
NCORES = 8
LN_EPS = 1e-5
BF16 = ml_dtypes.bfloat16
FP8 = ml_dtypes.float8_e4m3

_CACHE = {}

KC = D // 128      # 4 contraction chunks of 128
NCH = NS // 512    # 2 query-column chunks
MT = M // 128      # 16 key-position tiles
MCH = M // 512     # 4 key chunks of 512
MG = MT // 2       # 8 score groups (2 key tiles per group)
VW = 96            # fp8 V slot width (64 V + ones + 31 zeros), mult of 32
HWID = HD + 1      # bf16 V slot width (64 V + ones)

# exp shift (softmax-invariant; keeps exp in fp8 range)
ESH = 2.0
A16 = 128.0 / np.log(2.0)
# centered schraudolph bias incl. trunc->round (+0.5), rms centering
# (-7.4), and the -ESH shift
B0 = 127.0 * 128.0 + 0.5 - 7.4 - ESH * A16
BMASKED = B0 - 12288.0    # masked -> exp ~ 2^-100

AV_LAG = int(os.environ.get("AV_LAG", "3"))

# per-group exp path classes, g=0..7 (same for every ncc/head pair)
CLS = os.environ.get("CLS", "cbacbacb")
assert len(CLS) == MG and set(CLS) <= set("abc")
G_A = [g for g in range(MG) if CLS[g] == "a"]
G_B = [g for g in range(MG) if CLS[g] == "b"]
G_C = [g for g in range(MG) if CLS[g] == "c"]
NA, NB, NC = len(G_A), len(G_B), len(G_C)
SLOT = {}
for _lst in (G_A, G_B, G_C):
    for _i, _g in enumerate(_lst):
        SLOT[_g] = _i
# combined slot index for the shared bf16 V tile (classes a+b)
SLOTV = {g: i for i, g in enumerate(G_A + G_B)}


def _build(ln_affine=True):
    import concourse.bass as bass  # noqa: F401
    import concourse.tile as tile
    from concourse import bacc, library_config, mybir
    from concourse.masks import make_identity

    f32 = mybir.dt.float32
    bf16 = mybir.dt.bfloat16
    fp8 = mybir.dt.float8e4
    i16 = mybir.dt.int16
    Exp = mybir.ActivationFunctionType.Exp
    Sqrt = mybir.ActivationFunctionType.Sqrt
    sub = mybir.AluOpType.subtract
    mult = mybir.AluOpType.mult
    add = mybir.AluOpType.add
    DR = mybir.MatmulPerfMode.DoubleRow

    nc = bacc.Bacc(None, target_bir_lowering=False, debug=False)

    xqT_d = nc.dram_tensor("xqT", [D, NS], fp8, kind="ExternalInput")
    xkT_d = nc.dram_tensor("xkT", [D, M], fp8, kind="ExternalInput")
    xvT_d = nc.dram_tensor("xvT", [D, M], fp8, kind="ExternalInput")
    qres_d = nc.dram_tensor("qres", [NS, D], f32, kind="ExternalInput")
    wqT_d = nc.dram_tensor("wqT", [D, D], fp8, kind="ExternalInput")
    wkT_d = nc.dram_tensor("wkT", [D, D], fp8, kind="ExternalInput")
    wvT_d = nc.dram_tensor("wvT", [D, D], fp8, kind="ExternalInput")
    woT_d = nc.dram_tensor("woT", [D, D], fp8, kind="ExternalInput")
    gamma_d = nc.dram_tensor("gamma", [1, D], f32, kind="ExternalInput")
    beta_d = nc.dram_tensor("beta", [1, D], f32, kind="ExternalInput")
    if NA:
        maskA_d = nc.dram_tensor("maskA", [NCH * NA * 128, 1024], bf16,
                                 kind="ExternalInput")
    if NB:
        maskB_d = nc.dram_tensor("maskB", [NCH * NB * 128, 1024], f32,
                                 kind="ExternalInput")
    if NC:
        mask8_d = nc.dram_tensor("mask8", [NCH * NC * 128, 1024], fp8,
                                 kind="ExternalInput")
    out_d = nc.dram_tensor("out", [NS, D], f32, kind="ExternalOutput")

    with tile.TileContext(nc) as tc:
        with (
            tc.tile_pool(name="big", bufs=1) as big,
            tc.tile_pool(name="wpool", bufs=1) as wpool,
            tc.tile_pool(name="ppool", bufs=4) as ppool,
            tc.tile_pool(name="p8pool", bufs=4) as p8pool,
            tc.tile_pool(name="xpool", bufs=5) as xpool,
            tc.tile_pool(name="mvpool", bufs=6) as mvpool,
            tc.tile_pool(name="ypool", bufs=3) as ypool,
            tc.tile_pool(name="rpool", bufs=2) as rpool,
            tc.tile_pool(name="small", bufs=6) as small,
            tc.tile_pool(name="ps_mm", bufs=2, space="PSUM") as ps_mm,
            tc.tile_pool(name="ps_s", bufs=2, space="PSUM") as ps_s,
            tc.tile_pool(name="ps_o", bufs=1, space="PSUM") as ps_o,
            tc.tile_pool(name="pospool", bufs=2) as pospool,
        ):
            nc.gpsimd.load_library(library_config.proxy)

            # ---- resident SBUF tensors -----------------------------------
            xqT = big.tile([128, KC, NS], fp8, tag="xqT")
            xkT = big.tile([128, KC, M], fp8, tag="xkT")
            xvT = big.tile([128, KC, M], fp8, tag="xvT")
            qT = big.tile([128, KC, NS], bf16, tag="qT")
            kT = big.tile([128, KC, M], bf16, tag="kT")
            oT = big.tile([128, KC, NS], fp8, tag="oT")
            if NA:
                maskA = big.tile([128, NCH, NA, 1024], bf16, tag="maskA")
            if NB:
                maskB = big.tile([128, NCH, NB, 1024], f32, tag="maskB")
            if NC:
                mask8 = big.tile([128, NCH, NC, 1024], fp8, tag="mask8")
            # bf16 V slots (a/b groups): per (g, head): [2 keytiles, 65]
            nbf = NA + NB
            if nbf:
                vSb = big.tile([128, nbf, H, 2, HWID], bf16, tag="vSb")
            if NC:
                vS8 = big.tile([128, NC, H, 2, VW], fp8, tag="vS8")
            wq = wpool.tile([128, KC, D], fp8, tag="wq")
            wk = wpool.tile([128, KC, D], fp8, tag="wk")
            wv = wpool.tile([128, KC, D], fp8, tag="wv")
            wo = wpool.tile([128, KC, D], fp8, tag="wo")
            gamma_b = wpool.tile([128, D], f32, tag="gamma_b")
            beta_b = wpool.tile([128, D], f32, tag="beta_b")
            gamma_1 = wpool.tile([1, D], f32, tag="gamma_1")
            beta_1 = wpool.tile([1, D], f32, tag="beta_1")
            eps_t = wpool.tile([128, 1], f32, tag="eps")
            neg4_t = wpool.tile([128, 1], f32, tag="neg4")
            ident = wpool.tile([128, 128], f32, tag="ident")
            make_identity(nc, ident)

            # ---- setup ---------------------------------------------------
            nc.vector.memset(eps_t, LN_EPS)
            nc.vector.memset(neg4_t, -ESH)
            if nbf:
                nc.vector.memset(
                    vSb[:, :, :, :, HD : HD + 1], 1.0)
            if NC:
                nc.vector.memset(vS8[:, :, :, :, HD : HD + 1], 1.0)
                nc.vector.memset(vS8[:, :, :, :, HD + 1 :], 0.0)

            # ---- input DMAs, split per consumption chunk -----------------
            xq_r = xqT_d[:].rearrange("(c p) n -> p c n", p=128)
            xk_r = xkT_d[:].rearrange("(c p) n -> p c n", p=128)
            xv_r = xvT_d[:].rearrange("(c p) n -> p c n", p=128)
            if NA:
                mA_r = maskA_d[:].rearrange("(c s p) n -> p c s n", c=NCH, s=NA)
            if NB:
                mB_r = maskB_d[:].rearrange("(c s p) n -> p c s n", c=NCH, s=NB)
            if NC:
                m8_r = mask8_d[:].rearrange("(c s p) n -> p c s n", c=NCH, s=NC)

            def mask_dma(ncc, g):
                c = CLS[g]
                s = SLOT[g]
                if c == "a":
                    nc.sync.dma_start(out=maskA[:, ncc, s, :], in_=mA_r[:, ncc, s, :])
                elif c == "b":
                    nc.sync.dma_start(out=maskB[:, ncc, s, :], in_=mB_r[:, ncc, s, :])
                else:
                    nc.sync.dma_start(out=mask8[:, ncc, s, :], in_=m8_r[:, ncc, s, :])

            nc.sync.dma_start(out=wq, in_=wqT_d[:].rearrange("(c p) o -> p c o", p=128))
            for ncc in range(NCH):
                sl = slice(ncc * 512, (ncc + 1) * 512)
                nc.sync.dma_start(out=xqT[:, :, sl], in_=xq_r[:, :, sl])
            nc.sync.dma_start(out=wk, in_=wkT_d[:].rearrange("(c p) o -> p c o", p=128))
            for mc in range(MCH):
                sl = slice(mc * 512, (mc + 1) * 512)
                nc.sync.dma_start(out=xkT[:, :, sl], in_=xk_r[:, :, sl])
            mask_dma(0, 0)
            mask_dma(0, 1)
            nc.sync.dma_start(out=wv, in_=wvT_d[:].rearrange("(c p) o -> p c o", p=128))
            for jc in range(4):
                sl = slice(jc * 256, (jc + 1) * 256)
                nc.sync.dma_start(out=xvT[:, :, sl], in_=xv_r[:, :, sl])
            mask_dma(0, 2)
            mask_dma(0, 3)
            for jc in range(4, 8):
                sl = slice(jc * 256, (jc + 1) * 256)
                nc.sync.dma_start(out=xvT[:, :, sl], in_=xv_r[:, :, sl])
            for g in range(4, MG):
                mask_dma(0, g)
            nc.sync.dma_start(out=wo, in_=woT_d[:].rearrange("(c p) o -> p c o", p=128))
            for g in range(MG):
                mask_dma(1, g)
            nc.sync.dma_start(out=gamma_1, in_=gamma_d[:])
            nc.sync.dma_start(out=beta_1, in_=beta_d[:])
            nc.gpsimd.partition_broadcast(gamma_b, gamma_1, channels=128)
            nc.gpsimd.partition_broadcast(beta_b, beta_1, channels=128)

            # ---- projection emitters (fp8 DR matmuls; casts on DVE) ------
            def dr_proj(ps, lhsw, rhsx, csl0):
                # contraction 512 = 2 DR steps; columns 512 = 2 chunks
                for cch in range(2):
                    csl = slice(csl0 + cch * 256, csl0 + (cch + 1) * 256)
                    psl = slice(cch * 256, (cch + 1) * 256)
                    for s in range(2):
                        nc.tensor.matmul(
                            ps[:, psl],
                            lhsT=lhsw(s),
                            rhs=rhsx(s, csl),
                            start=(s == 0),
                            stop=(s == 1),
                            perf_mode=DR,
                        )

            def q_proj(t, ncc):
                ps = ps_mm.tile([128, 512], f32, tag="mm")
                tb = slice(t * 128, (t + 1) * 128)
                dr_proj(
                    ps,
                    lambda s: wq[:, 2 * s : 2 * s + 2, tb],
                    lambda s, csl: xqT[:, 2 * s : 2 * s + 2, csl],
                    ncc * 512,
                )
                nc.vector.tensor_copy(
                    out=qT[:, t, ncc * 512 : (ncc + 1) * 512], in_=ps)

            def k_proj(t, mc):
                ps = ps_mm.tile([128, 512], f32, tag="mm")
                tb = slice(t * 128, (t + 1) * 128)
                dr_proj(
                    ps,
                    lambda s: wk[:, 2 * s : 2 * s + 2, tb],
                    lambda s, csl: xkT[:, 2 * s : 2 * s + 2, csl],
                    mc * 512,
                )
                nc.vector.tensor_copy(
                    out=kT[:, t, mc * 512 : (mc + 1) * 512], in_=ps)

            def v_proj(j):
                # V[key tile j, all 512 dims] -> class-dependent slot store
                ps = ps_mm.tile([128, 512], f32, tag="mm")
                jb = slice(j * 128, (j + 1) * 128)
                dr_proj(
                    ps,
                    lambda s: xvT[:, 2 * s : 2 * s + 2, jb],
                    lambda s, csl: wv[:, 2 * s : 2 * s + 2, csl],
                    0,
                )
                g, u = j // 2, j % 2
                c = CLS[g]
                if c in "ab":
                    dst = vSb[:, SLOTV[g], :, u, 0:HD]
                else:
                    dst = vS8[:, SLOT[g], :, u, 0:HD]
                nc.vector.tensor_copy(
                    out=dst, in_=ps[:].rearrange("p (h x) -> p h x", x=HD))

            # ---- normalize -----------------------------------------------
            # po PSUM is staged to SBUF with ONE wide ACT copy (frees the
            # PSUM bank ~immediately -> ps_o bufs=1 suffices); the recip
            # dance + muls then run from SBUF.
            def normalize_pair(poS, dS, t, nsl, flat=False):
                # poS [64, 1024] f32 SBUF: head-even cols 0:512, odd 512:1024
                # dS [1, 1024]: the two heads' softmax denominators
                if flat:
                    recip_s = rpool.tile([1, 1024], f32, tag="recip")
                    nc.vector.reciprocal(recip_s, dS)
                else:
                    scr = ps_mm.tile([128, 512], f32, tag="mm")
                    dT = scr[:, 0:8]
                    rrow = scr[0:1, 0:512]
                    for c in range(8):
                        nc.tensor.transpose(
                            dT[:, c : c + 1],
                            dS[:, c * 128 : (c + 1) * 128],
                            ident[0:1, 0:1],
                        )
                    rT = small.tile([128, 8], f32, tag="rT")
                    nc.vector.reciprocal(rT, dT)
                    recip_s = rpool.tile([1, 1024], f32, tag="recip")
                    for half in range(2):
                        for c in range(4):
                            nc.tensor.transpose(
                                rrow[:, c * 128 : (c + 1) * 128],
                                rT[:, 4 * half + c : 4 * half + c + 1], ident
                            )
                        nc.scalar.copy(
                            out=recip_s[:, half * 512 : (half + 1) * 512],
                            in_=rrow)
                rb = rpool.tile([64, 1024], f32, tag="rb")
                nc.gpsimd.partition_broadcast(rb, recip_s, channels=64)
                nc.vector.tensor_mul(
                    oT[0:64, t, nsl], poS[:, 0:512], rb[:, 0:512])
                nc.vector.tensor_mul(
                    oT[64:128, t, nsl], poS[:, 512:1024], rb[:, 512:1024])

            # ---- attention stream ----------------------------------------
            # unit = (t, ncc, g, h): ONE head of one score group.  Score
            # PSUM double-buffered (bufs=2) so unit i+1's matmuls never
            # wait on unit i's exp (the WAR chain that starved the PE).
            PO_P = VW if NC else HWID
            pend = {}   # (t, ncc) -> (po, nsl)
            pts = {}    # unit -> pt handle

            def emit_av(unit):
                t, ncc, g, h = unit
                po, _ = pend[(t, ncc)]
                c = CLS[g]
                s_ = SLOT[g]
                pX = pts.pop(unit)
                first = g == 0
                last = g == MG - 1
                if c == "c":
                    rhs = pX[:].rearrange("p (s n) -> p s n", s=2)
                    for cch in range(2):
                        csl = slice(cch * 256, (cch + 1) * 256)
                        osl = slice(h * 512 + cch * 256,
                                    h * 512 + (cch + 1) * 256)
                        nc.tensor.matmul(
                            po[:, osl], lhsT=vS8[:, s_, 2 * t + h, :, :],
                            rhs=rhs[:, :, csl],
                            start=first, stop=last,
                            perf_mode=DR, skip_group_check=True,
                        )
                else:
                    rhsF = pX[:].bitcast(bf16) if c == "b" else pX[:]
                    for u in range(2):
                        usl = slice(u * 512, (u + 1) * 512)
                        osl = slice(h * 512, (h + 1) * 512)
                        nc.tensor.matmul(
                            po[0:HWID, osl],
                            lhsT=vSb[:, SLOTV[g], 2 * t + h, u, :],
                            rhs=rhsF[:, usl],
                            start=(first and u == 0),
                            stop=(last and u == 1),
                            skip_group_check=True,
                        )
                if last and h == 1:
                    po, nsl = pend.pop((t, ncc))
                    poS = pospool.tile([HD, 1024], f32, tag="poS")
                    nc.scalar.copy(out=poS, in_=po[0:HD, :])
                    dS = rpool.tile([1, 1024], f32, tag="dS")
                    nc.scalar.copy(out=dS, in_=po[HD : HD + 1, :])
                    normalize_pair(poS, dS, t, nsl,
                                   flat=(ncc == 1 and t >= 2))

            def attend_all(pair_order, fillmap):
                units = [(t, ncc, g, h) for (t, ncc) in pair_order
                         for g in range(MG) for h in range(2)]
                from collections import deque
                lagq = deque()
                for unit in units:
                    t, ncc, g, h = unit
                    nsl = slice(ncc * 512, (ncc + 1) * 512)
                    if g == 0 and h == 0:
                        po_new = ps_o.tile([PO_P, 1024], f32, tag="po")
                        pend[(t, ncc)] = (po_new, nsl)
                    ps = ps_s.tile([128, 1024], f32, tag="s")
                    hsl = slice(h * 64, (h + 1) * 64)
                    for u in range(2):
                        j = 2 * g + u
                        usl = slice(u * 512, (u + 1) * 512)
                        nc.tensor.matmul(
                            ps[:, usl],
                            lhsT=kT[hsl, t, j * 128 : (j + 1) * 128],
                            rhs=qT[hsl, t, nsl],
                            start=True, stop=True,
                        )
                    c = CLS[g]
                    s_ = SLOT[g]
                    if c == "a":
                        pt = ppool.tile([128, 1024], bf16, tag="pt")
                        nc.scalar.activation(pt, ps, Exp, bias=neg4_t,
                                             scale=0.125)
                        nc.vector.tensor_mul(pt, pt, maskA[:, ncc, s_, :])
                        pts[unit] = pt
                    elif c == "b":
                        pt = ppool.tile([128, 1024], i16, tag="pti")
                        nc.vector.scalar_tensor_tensor(
                            out=pt, in0=ps, scalar=float(A16 * 0.125),
                            in1=maskB[:, ncc, s_, :], op0=mult, op1=add)
                        pts[unit] = pt
                    else:
                        pt0 = p8pool.tile([128, 1024], fp8, tag="pt8")
                        nc.scalar.activation(pt0, ps, Exp, bias=neg4_t,
                                             scale=0.125)
                        pt = p8pool.tile([128, 1024], fp8, tag="pm8")
                        nc.gpsimd.tensor_mul(pt, pt0, mask8[:, ncc, s_, :])
                        pts[unit] = pt
                    if h == 0:
                        for f in fillmap.get((t, ncc), {}).get(g, ()):
                            f()
                    lagq.append(unit)
                    if len(lagq) > AV_LAG:
                        emit_av(lagq.popleft())
                while lagq:
                    emit_av(lagq.popleft())

            # ---- output projection + residual + LayerNorm ----------------
            qres_r = qres_d[:].rearrange("(t p) d -> p t d", p=128)
            out_r = out_d[:].rearrange("(t p) d -> p t d", p=128)
            ot_state = {}

            def out_front(nt):
                ps = ps_mm.tile([128, 512], f32, tag="mm")
                ntb = slice(nt * 128, (nt + 1) * 128)
                dr_proj(
                    ps,
                    lambda s: oT[:, 2 * s : 2 * s + 2, ntb],
                    lambda s, csl: wo[:, 2 * s : 2 * s + 2, csl],
                    0,
                )
                qres_t = ypool.tile([128, D], f32, tag="qres")
                nc.sync.dma_start(out=qres_t, in_=qres_r[:, nt, :])
                x_t = xpool.tile([128, D], f32, tag="x")
                nc.vector.tensor_add(x_t, ps, qres_t)
                stats = small.tile([128, 6], f32, tag="stats")
                nc.vector.bn_stats(out=stats, in_=x_t)
                mv = mvpool.tile([128, 2], f32, tag="mv")
                nc.vector.bn_aggr(out=mv, in_=stats)
                ot_state[nt] = (x_t, mv)

            rstd_store = {}

            def rstd_batch(nts):
                vcol = small.tile([128, 4], f32, tag="vcol")
                for i, nt in enumerate(nts):
                    nc.vector.tensor_copy(out=vcol[:, i : i + 1],
                                          in_=ot_state[nt][1][:, 1:2])
                sd = small.tile([128, 4], f32, tag="sd")
                nc.scalar.activation(sd, vcol, Sqrt, bias=eps_t)
                rs = mvpool.tile([128, 4], f32, tag="rs")
                nc.vector.reciprocal(rs, sd)
                for i, nt in enumerate(nts):
                    rstd_store[nt] = (rs, i)

            def out_back(nt, tail=False):
                x_t, mv = ot_state.pop(nt)
                rs, i = rstd_store.pop(nt)
                xn = ypool.tile([128, D], f32, tag="xn")
                nc.vector.tensor_scalar(
                    out=xn, in0=x_t, scalar1=mv[:, 0:1], scalar2=rs[:, i : i + 1],
                    op0=sub, op1=mult,
                )
                if ln_affine:
                    y_t = ypool.tile([128, D], f32, tag="y")
                    nc.vector.tensor_mul(y_t, xn, gamma_b)
                    nc.vector.tensor_add(y_t, y_t, beta_b)
                else:
                    y_t = xn
                nc.sync.dma_start(out=out_r[:, nt, :], in_=y_t)

            # ---- emission schedule ---------------------------------------
            q_proj(0, 0)
            q_proj(0, 1)
            for mc in range(MCH):
                k_proj(0, mc)
            v_proj(0)
            v_proj(1)

            def C(f, *a):
                return lambda: f(*a)

            f00 = {
                0: (C(v_proj, 2), C(v_proj, 3)),
                1: (C(v_proj, 4), C(v_proj, 5)),
                2: (C(v_proj, 6), C(v_proj, 7)),
                3: (C(v_proj, 8), C(v_proj, 9)),
                4: (C(v_proj, 10), C(v_proj, 11)),
                5: (C(v_proj, 12), C(v_proj, 13)),
                6: (C(v_proj, 14), C(v_proj, 15), C(q_proj, 1, 0)),
                7: (C(q_proj, 1, 1), C(k_proj, 1, 0)),
            }
            f10 = {
                0: (C(k_proj, 1, 1),),
                1: (C(k_proj, 1, 2), C(k_proj, 1, 3)),
                3: (C(q_proj, 2, 0),),
                4: (C(q_proj, 2, 1),),
                5: (C(k_proj, 2, 0),),
                6: (C(k_proj, 2, 1),),
                7: (C(k_proj, 2, 2), C(k_proj, 2, 3)),
            }
            f20 = {
                0: (C(q_proj, 3, 0),),
                1: (C(q_proj, 3, 1),),
                4: (C(k_proj, 3, 0),),
                5: (C(k_proj, 3, 1),),
                6: (C(k_proj, 3, 2), C(k_proj, 3, 3)),
            }
            f01 = {2: (C(out_front, 0),), 4: (C(out_front, 1),),
                   6: (C(out_front, 2),)}
            f11 = {0: (C(out_front, 3),)}
            f21 = {4: (C(rstd_batch, (0, 1, 2, 3)),)}
            f31 = {0: (C(out_back, 0),), 2: (C(out_back, 1),),
                   4: (C(out_back, 2),), 6: (C(out_back, 3),)}

            pair_order = [(0, 0), (1, 0), (2, 0), (3, 0),
                          (0, 1), (1, 1), (2, 1), (3, 1)]
            fillmap = {(0, 0): f00, (1, 0): f10, (2, 0): f20,
                       (0, 1): f01, (1, 1): f11, (2, 1): f21, (3, 1): f31}
            attend_all(pair_order, fillmap)
            out_front(4)
            out_front(5)
            out_front(6)
            out_front(7)
            rstd_batch((4, 5, 6, 7))
            out_back(4, tail=True)
            out_back(5, tail=True)
            out_back(6, tail=True)
            out_back(7, tail=True)

    nc.compile()
    return nc


def kernel(**inputs):
    from concourse.bass_utils import run_bass_kernel_spmd

    gamma_a = np.asarray(inputs["gamma"], dtype=np.float32)
    beta_a = np.asarray(inputs["beta"], dtype=np.float32)
    ln_affine = bool(np.any(gamma_a != 1.0) or np.any(beta_a != 0.0))
    ck = ("nc", ln_affine, CLS)
    if ck not in _CACHE:
        _CACHE[ck] = _build(ln_affine)
    nc = _CACHE[ck]

    query = np.asarray(inputs["query"], dtype=np.float32)
    key = np.asarray(inputs["key"], dtype=np.float32)
    value = np.asarray(inputs["value"], dtype=np.float32)
    mask = np.asarray(inputs["mask"])
    WQ = np.asarray(inputs["WQ"], dtype=np.float32)
    WK = np.asarray(inputs["WK"], dtype=np.float32)
    WV = np.asarray(inputs["WV"], dtype=np.float32)
    WO = np.asarray(inputs["WO"], dtype=np.float32)
    bO = np.asarray(inputs["bO"], dtype=np.float32)
    gamma = np.asarray(inputs["gamma"], dtype=np.float32)
    beta = np.asarray(inputs["beta"], dtype=np.float32)

    scale = np.float32(1.0 / np.sqrt(HD))
    wqT = np.ascontiguousarray(WQ.T).astype(FP8)
    wkT = np.ascontiguousarray(WK.T).astype(FP8)
    wvT = np.ascontiguousarray(WV.T).astype(FP8)
    woT = np.ascontiguousarray(WO.T).astype(FP8)
    gamma_in = gamma.reshape(1, D)
    beta_in = beta.reshape(1, D)
    mask_bin = (mask != 0)

    in_maps = []
    for c in range(NCORES):
        b, n0 = c // 2, (c % 2) * NS
        mT = np.ascontiguousarray(mask_bin[b, n0 : n0 + NS, :].T)  # [M, NS]
        mP = (
            mT.reshape(8, 2, 128, 2, 512)
            .transpose(3, 0, 2, 1, 4)
            .reshape(2, 8, 128, 1024)
        )
        im = {
            "xqT": np.ascontiguousarray(query[b, n0 : n0 + NS, :].T).astype(FP8),
            "xkT": np.ascontiguousarray(key[b].T).astype(FP8),
            "xvT": np.ascontiguousarray(value[b].T).astype(FP8),
            "qres": np.ascontiguousarray(query[b, n0 : n0 + NS, :] + bO[None, :]),
            "wqT": wqT, "wkT": wkT, "wvT": wvT, "woT": woT,
            "gamma": gamma_in, "beta": beta_in,
        }
        if NA:
            ma = mP[:, G_A].astype(BF16)
            im["maskA"] = np.ascontiguousarray(ma.reshape(-1, 1024))
        if NB:
            mb_ = np.where(mP[:, G_B], np.float32(B0), np.float32(BMASKED))
            im["maskB"] = np.ascontiguousarray(
                mb_.astype(np.float32).reshape(-1, 1024))
        if NC:
            m8 = mP[:, G_C].astype(FP8)
            im["mask8"] = np.ascontiguousarray(m8.reshape(-1, 1024))
        in_maps.append(im)

    trace = bool(int(os.environ.get("BASS_KERNEL_TRACE", "0")))
    res = run_bass_kernel_spmd(nc, in_maps, core_ids=list(range(NCORES)), trace=trace)
    _CACHE["last_results"] = res

    out = np.empty((B, N, D), dtype=np.float32)
    for c in range(NCORES):
        b, n0 = c // 2, (c % 2) * NS
        out[b, n0 : n0 + NS, :] = res.results[c]["out"]
    return out


# revision 14
# speedup vs baseline: 1.3639x; 1.3639x over previous
"""MultiHeadGraphAttention TRN2 kernel, v2.

Data-parallel over (batch, query-half): core c handles batch c//2, query rows
(c%2)*1024 .. +1024.  All matmuls bf16 (fp32 PSUM); softmax + LayerNorm fp32.

v2 changes vs baseline (337us):
 - ScalarE is the wall (~130us of exp).  Everything else is arranged to hide
   under it: PSUM->SBUF projection copies moved to DVE, LayerNorm rstd uses
   ln+exp (both in the natural_log_exp_and_others table set -> no table
   thrash; Sqrt previously forced 10 table reloads mid-kernel and stalled the
   exp stream).
 - Score matmuls of a head PAIR run concurrently on disjoint PE row halves
   (K=64 each; tile_position auto-derived from base partitions 0/64).
 - Attention inner loop is software-pipelined: AV matmuls of group g-1 are
   emitted after the score matmuls of group g, so the in-order PE queue never
   blocks the next score tile (and the exp stream) behind a mask-waiting AV.
 - Input DMAs are split per consumption chunk and emitted in consumption
   order; projections start as soon as their inputs land (~4us) instead of
   after all input DMA (~38us).  Remaining projections are threaded into the
   attention stream as PE filler so the PE never idles > ~1us (HAM stays at
   K=8/8).
 - softmax denominator from an appended ones-column on V (row 64 of the AV
   output); reciprocal on DVE, partition-broadcast + normalize mul on GPSIMD.
"""

import os
import sys

import numpy as np

try:
    import concourse  # noqa: F401
except ImportError:  # harness runs from a bare dir; the repo is a fixed path
    sys.path.insert(0, "/opt/trn_rl_repo")

import ml_dtypes

B, N, M, D, H, HD = 4, 2048, 2048, 512, 8, 64
NS = 1024          # query rows per core
NCORES = 8
LN_EPS = 1e-5
BF16 = ml_dtypes.bfloat16

_CACHE = {}

# fallback knobs (read once at build)
# NOTE: reciprocal_approx_fast passes CoreSim but returns garbage on HW.
# NOTE: GPSIMD cannot access PSUM (BIR verifier) -> PSUM-reading ops on DVE.
K_XT = int(os.environ.get("K_XT", "0"))   # x_t add on gpsimd vs vector


def _build(ln_affine=True):
    import concourse.bass as bass  # noqa: F401
    import concourse.tile as tile
    from concourse import bacc, mybir
    from concourse.masks import make_identity

    f32 = mybir.dt.float32
    bf16 = mybir.dt.bfloat16
    Exp = mybir.ActivationFunctionType.Exp
    Sqrt = mybir.ActivationFunctionType.Sqrt
    sub = mybir.AluOpType.subtract
    mult = mybir.AluOpType.mult

    nc = bacc.Bacc(None, target_bir_lowering=False, debug=False)

    xqT_d = nc.dram_tensor("xqT", [D, NS], bf16, kind="ExternalInput")
    xkT_d = nc.dram_tensor("xkT", [D, M], bf16, kind="ExternalInput")
    xvT_d = nc.dram_tensor("xvT", [D, M], bf16, kind="ExternalInput")
    maskP_d = nc.dram_tensor("maskP", [2 * 8 * 128, 1024], bf16, kind="ExternalInput")
    qres_d = nc.dram_tensor("qres", [NS, D], f32, kind="ExternalInput")
    wqT_d = nc.dram_tensor("wqT", [D, D], bf16, kind="ExternalInput")
    wkT_d = nc.dram_tensor("wkT", [D, D], bf16, kind="ExternalInput")
    wvT_d = nc.dram_tensor("wvT", [D, D], bf16, kind="ExternalInput")
    woT_d = nc.dram_tensor("woT", [D, D], bf16, kind="ExternalInput")
    gamma_d = nc.dram_tensor("gamma", [1, D], f32, kind="ExternalInput")
    beta_d = nc.dram_tensor("beta", [1, D], f32, kind="ExternalInput")
    out_d = nc.dram_tensor("out", [NS, D], f32, kind="ExternalOutput")

    KC = D // 128      # 4 contraction chunks of 128
    NCH = NS // 512    # 2 query-column chunks
    MT = M // 128      # 16 key-position tiles
    MCH = M // 512     # 4 key chunks of 512
    MG = MT // 2       # 8 score groups (2 key tiles per group)
    HW = HD + 1        # per-head V slot width (64 V cols + ones col)

    with tile.TileContext(nc) as tc:
        with (
            tc.tile_pool(name="big", bufs=1) as big,
            tc.tile_pool(name="wpool", bufs=1) as wpool,
            tc.tile_pool(name="ppool", bufs=4) as ppool,
            tc.tile_pool(name="xpool", bufs=5) as xpool,
            tc.tile_pool(name="mvpool", bufs=6) as mvpool,
            tc.tile_pool(name="ypool", bufs=3) as ypool,
            tc.tile_pool(name="rpool", bufs=2) as rpool,
            tc.tile_pool(name="small", bufs=6) as small,
            tc.tile_pool(name="ps_mm", bufs=2, space="PSUM") as ps_mm,
            tc.tile_pool(name="ps_sA", bufs=1, space="PSUM") as ps_sA,
            tc.tile_pool(name="ps_sB", bufs=1, space="PSUM") as ps_sB,
            tc.tile_pool(name="ps_o", bufs=1, space="PSUM") as ps_o,
        ):
            # ---- resident SBUF tensors -----------------------------------
            xqT = big.tile([128, KC, NS], bf16, tag="xqT")
            xkT = big.tile([128, KC, M], bf16, tag="xkT")
            xvT = big.tile([128, KC, M], bf16, tag="xvT")
            maskS = big.tile([128, NCH, MG, 1024], bf16, tag="maskS")
            qT = big.tile([128, KC, NS], bf16, tag="qT")
            kT = big.tile([128, KC, M], bf16, tag="kT")
            vS = big.tile([128, MT, H * HW], bf16, tag="vS")
            oT = big.tile([128, KC, NS], bf16, tag="oT")
            wq = wpool.tile([128, KC, D], bf16, tag="wq")
            wk = wpool.tile([128, KC, D], bf16, tag="wk")
            wv = wpool.tile([128, KC, D], bf16, tag="wv")
            wo = wpool.tile([128, KC, D], bf16, tag="wo")
            gamma_b = wpool.tile([128, D], f32, tag="gamma_b")
            beta_b = wpool.tile([128, D], f32, tag="beta_b")
            gamma_1 = wpool.tile([1, D], f32, tag="gamma_1")
            beta_1 = wpool.tile([1, D], f32, tag="beta_1")
            eps_t = wpool.tile([128, 1], f32, tag="eps")
            ident = wpool.tile([128, 128], f32, tag="ident")
            make_identity(nc, ident)

            # ---- setup (no DMA dependencies; engines idle early) ---------
            nc.vector.memset(eps_t, LN_EPS)
            # ones column per head in the augmented V (softmax denominator
            # lands as row 64 of the AV matmul output)
            nc.vector.memset(
                vS[:].rearrange("p j (h x) -> p j h x", x=HW)[:, :, :, HD : HD + 1],
                1.0,
            )

            # ---- input DMAs, split per consumption chunk, priority order -
            xq_r = xqT_d[:].rearrange("(c p) n -> p c n", p=128)
            xk_r = xkT_d[:].rearrange("(c p) n -> p c n", p=128)
            xv_r = xvT_d[:].rearrange("(c p) n -> p c n", p=128)
            mk_r = maskP_d[:].rearrange("(c g p) n -> p c g n", c=NCH, g=MG)

            nc.sync.dma_start(out=wq, in_=wqT_d[:].rearrange("(c p) o -> p c o", p=128))
            for ncc in range(NCH):
                sl = slice(ncc * 512, (ncc + 1) * 512)
                nc.sync.dma_start(out=xqT[:, :, sl], in_=xq_r[:, :, sl])
            nc.sync.dma_start(out=wk, in_=wkT_d[:].rearrange("(c p) o -> p c o", p=128))
            for mc in range(MCH):
                sl = slice(mc * 512, (mc + 1) * 512)
                nc.sync.dma_start(out=xkT[:, :, sl], in_=xk_r[:, :, sl])
            nc.sync.dma_start(out=maskS[:, 0, 0, :], in_=mk_r[:, 0, 0, :])
            nc.sync.dma_start(out=maskS[:, 0, 1, :], in_=mk_r[:, 0, 1, :])
            nc.sync.dma_start(out=wv, in_=wvT_d[:].rearrange("(c p) o -> p c o", p=128))
            for jc in range(4):
                sl = slice(jc * 256, (jc + 1) * 256)
                nc.sync.dma_start(out=xvT[:, :, sl], in_=xv_r[:, :, sl])
            nc.sync.dma_start(out=maskS[:, 0, 2, :], in_=mk_r[:, 0, 2, :])
            nc.sync.dma_start(out=maskS[:, 0, 3, :], in_=mk_r[:, 0, 3, :])
            for jc in range(4, 8):
                sl = slice(jc * 256, (jc + 1) * 256)
                nc.sync.dma_start(out=xvT[:, :, sl], in_=xv_r[:, :, sl])
            for g in range(4, MG):
                nc.sync.dma_start(out=maskS[:, 0, g, :], in_=mk_r[:, 0, g, :])
            nc.sync.dma_start(out=wo, in_=woT_d[:].rearrange("(c p) o -> p c o", p=128))
            for g in range(MG):
                nc.sync.dma_start(out=maskS[:, 1, g, :], in_=mk_r[:, 1, g, :])
            nc.sync.dma_start(out=gamma_1, in_=gamma_d[:])
            nc.sync.dma_start(out=beta_1, in_=beta_d[:])
            nc.gpsimd.partition_broadcast(gamma_b, gamma_1, channels=128)
            nc.gpsimd.partition_broadcast(beta_b, beta_1, channels=128)

            # ---- projection emitters (PSUM->SBUF copies on DVE) ----------
            def q_proj(t, ncc):
                ps = ps_mm.tile([128, 512], f32, tag="mm")
                sl = slice(ncc * 512, (ncc + 1) * 512)
                for kc in range(KC):
                    nc.tensor.matmul(
                        ps,
                        lhsT=wq[:, kc, t * 128 : (t + 1) * 128],
                        rhs=xqT[:, kc, sl],
                        start=(kc == 0),
                        stop=(kc == KC - 1),
                    )
                nc.vector.tensor_copy(out=qT[:, t, sl], in_=ps)

            def k_proj(t, mc):
                ps = ps_mm.tile([128, 512], f32, tag="mm")
                sl = slice(mc * 512, (mc + 1) * 512)
                for kc in range(KC):
                    nc.tensor.matmul(
                        ps,
                        lhsT=wk[:, kc, t * 128 : (t + 1) * 128],
                        rhs=xkT[:, kc, sl],
                        start=(kc == 0),
                        stop=(kc == KC - 1),
                    )
                nc.vector.tensor_copy(out=kT[:, t, sl], in_=ps)

            def v_proj(j):
                # V[m, o] straight, scattered into per-head 65-wide slots
                ps = ps_mm.tile([128, 512], f32, tag="mm")
                for kc in range(KC):
                    nc.tensor.matmul(
                        ps,
                        lhsT=xvT[:, kc, j * 128 : (j + 1) * 128],
                        rhs=wv[:, kc, :],
                        start=(kc == 0),
                        stop=(kc == KC - 1),
                    )
                nc.vector.tensor_copy(
                    out=vS[:, j, :].rearrange("p (h x) -> p h x", x=HW)[:, :, 0:HD],
                    in_=ps[:].rearrange("p (h x) -> p h x", x=HD),
                )

            # ---- attention: head pair 2t/2t+1, software-pipelined --------
            # GPSIMD ucode note: partition_broadcast and tensor ops live in
            # DIFFERENT gpsimd libraries; alternating them costs a ~5us
            # UNLOAD_LIB/LOAD_LIB pair each time.  GPSIMD therefore runs
            # ONLY partition_broadcast; every tensor op goes to DVE.
            def normalize_flat(po_t, h, t, nsl):
                # latency-optimized variant for the final pairs: 4 queue hops
                # instead of 7.  The 3us one-lane reciprocal is fine when the
                # only consumer is the kernel tail.
                po2 = (h % 2) * 64
                dS = rpool.tile([1, 512], f32, tag="dS")
                nc.vector.tensor_copy(out=dS, in_=po_t[HD : HD + 1, :])
                recip_s = rpool.tile([1, 512], f32, tag="recip")
                nc.vector.reciprocal(recip_s, dS)
                rb = rpool.tile([64, 512], f32, tag="rb")
                nc.gpsimd.partition_broadcast(rb, recip_s, channels=64)
                nc.vector.tensor_mul(oT[po2 : po2 + 64, t, nsl], po_t[0:HD, :], rb)

            def normalize(po_t, h, t, nsl):
                # reciprocal via the PE-transpose dance — DVE reciprocal is
                # ~6 cycles/elem along the FREE dim, so [128,4] (0.2us)
                # beats [1,512] (3us)
                po2 = (h % 2) * 64
                dS = rpool.tile([1, 512], f32, tag="dS")
                nc.vector.tensor_copy(out=dS, in_=po_t[HD : HD + 1, :])
                scr = ps_mm.tile([128, 512], f32, tag="mm")
                dT = scr[:, 0:4]
                rrow = scr[0:1, 0:512]
                for c in range(KC):
                    nc.tensor.transpose(
                        dT[:, c : c + 1], dS[:, c * 128 : (c + 1) * 128],
                        ident[0:1, 0:1],
                    )
                rT = small.tile([128, 4], f32, tag="rT")
                nc.vector.reciprocal(rT, dT)
                for c in range(KC):
                    nc.tensor.transpose(
                        rrow[:, c * 128 : (c + 1) * 128], rT[:, c : c + 1], ident
                    )
                recip_s = rpool.tile([1, 512], f32, tag="recip")
                nc.vector.tensor_copy(out=recip_s, in_=rrow)
                rb = rpool.tile([64, 512], f32, tag="rb")
                nc.gpsimd.partition_broadcast(rb, recip_s, channels=64)
                nc.vector.tensor_mul(oT[po2 : po2 + 64, t, nsl], po_t[0:HD, :], rb)

            # one continuous stream over all (pair, chunk, group) units; the
            # AV matmuls run one unit behind the score/exp/mask front so the
            # in-order PE queue never blocks the exp stream, and the pipeline
            # never drains at pair boundaries
            pend = {}   # live pair state: (t, ncc) -> (poE, poO, nsl)
            pts = {}    # unit -> (ptA, ptB)

            def emit_av(unit):
                t, ncc, g = unit
                poE, poO, _ = pend[(t, ncc)]
                slotE = slice((2 * t) * HW, (2 * t + 1) * HW)
                slotO = slice((2 * t + 1) * HW, (2 * t + 2) * HW)
                ptA, ptB = pts.pop(unit)
                for u in range(2):
                    j = 2 * g + u
                    usl = slice(u * 512, (u + 1) * 512)
                    nc.tensor.matmul(
                        poE, lhsT=vS[:, j, slotE], rhs=ptA[:, usl],
                        start=(j == 0), stop=(j == MT - 1),
                    )
                    nc.tensor.matmul(
                        poO, lhsT=vS[:, j, slotO], rhs=ptB[:, usl],
                        start=(j == 0), stop=(j == MT - 1),
                    )
                if g == MG - 1:
                    poE, poO, nsl = pend.pop((t, ncc))
                    norm = normalize_flat if (ncc == 1 and t >= 2) else normalize
                    norm(poE, 2 * t, t, nsl)
                    norm(poO, 2 * t + 1, t, nsl)

            def attend_all(pair_order, fillmap):
                units = [(t, ncc, g) for (t, ncc) in pair_order for g in range(MG)]
                prev = None
                for unit in units:
                    t, ncc, g = unit
                    nsl = slice(ncc * 512, (ncc + 1) * 512)
                    if g == 0:
                        poE_new = ps_o.tile([HW, 512], f32, tag="poE")
                        poO_new = ps_o.tile([HW, 512], f32, tag="poO")
                        pend[(t, ncc)] = (poE_new, poO_new, nsl)
                    poE, poO, _ = pend[(t, ncc)]
                    psA = ps_sA.tile([128, 1024], f32, tag="sA")
                    psB = ps_sB.tile([128, 1024], f32, tag="sB")
                    for u in range(2):
                        j = 2 * g + u
                        usl = slice(u * 512, (u + 1) * 512)
                        # two heads on disjoint PE row halves -> concurrent
                        nc.tensor.matmul(
                            psA[:, usl],
                            lhsT=kT[0:64, t, j * 128 : (j + 1) * 128],
                            rhs=qT[0:64, t, nsl],
                            start=True, stop=True,
                        )
                        nc.tensor.matmul(
                            psB[:, usl],
                            lhsT=kT[64:128, t, j * 128 : (j + 1) * 128],
                            rhs=qT[64:128, t, nsl],
                            start=True, stop=True,
                        )
                    ptA = ppool.tile([128, 1024], bf16, tag="pt")
                    nc.scalar.activation(ptA, psA, Exp)
                    ptB = ppool.tile([128, 1024], bf16, tag="pt")
                    nc.scalar.activation(ptB, psB, Exp)
                    nc.vector.tensor_mul(ptA, ptA, maskS[:, ncc, g, :])
                    nc.vector.tensor_mul(ptB, ptB, maskS[:, ncc, g, :])
                    pts[unit] = (ptA, ptB)
                    for f in fillmap.get((t, ncc), {}).get(g, ()):
                        f()
                    if prev is not None:
                        emit_av(prev)
                    prev = unit
                emit_av(prev)

            # ---- output projection + residual + LayerNorm ----------------
            qres_r = qres_d[:].rearrange("(t p) d -> p t d", p=128)
            out_r = out_d[:].rearrange("(t p) d -> p t d", p=128)
            ot_state = {}

            def out_front(nt):
                ps = ps_mm.tile([128, 512], f32, tag="mm")
                for a in range(KC):
                    nc.tensor.matmul(
                        ps,
                        lhsT=oT[:, a, nt * 128 : (nt + 1) * 128],
                        rhs=wo[:, a, :],
                        start=(a == 0),
                        stop=(a == KC - 1),
                    )
                qres_t = ypool.tile([128, D], f32, tag="qres")
                nc.sync.dma_start(out=qres_t, in_=qres_r[:, nt, :])
                x_t = xpool.tile([128, D], f32, tag="x")
                if K_XT:
                    nc.gpsimd.tensor_add(x_t, ps, qres_t)
                else:
                    nc.vector.tensor_add(x_t, ps, qres_t)
                stats = small.tile([128, 6], f32, tag="stats")
                nc.vector.bn_stats(out=stats, in_=x_t)
                mv = mvpool.tile([128, 2], f32, tag="mv")
                nc.vector.bn_aggr(out=mv, in_=stats)
                ot_state[nt] = (x_t, mv)

            rstd_store = {}

            def rstd_batch(nts):
                # one Sqrt activation for a wave of tiles -> 2 ACT table
                # switches per wave instead of 2 per tile
                vcol = small.tile([128, 4], f32, tag="vcol")
                for i, nt in enumerate(nts):
                    nc.vector.tensor_copy(out=vcol[:, i : i + 1],
                                          in_=ot_state[nt][1][:, 1:2])
                sd = small.tile([128, 4], f32, tag="sd")
                nc.scalar.activation(sd, vcol, Sqrt, bias=eps_t)
                rs = mvpool.tile([128, 4], f32, tag="rs")
                nc.vector.reciprocal(rs, sd)
                for i, nt in enumerate(nts):
                    rstd_store[nt] = (rs, i)

            def out_back(nt, tail=False):
                x_t, mv = ot_state.pop(nt)
                rs, i = rstd_store.pop(nt)
                xn = ypool.tile([128, D], f32, tag="xn")
                nc.vector.tensor_scalar(
                    out=xn, in0=x_t, scalar1=mv[:, 0:1], scalar2=rs[:, i : i + 1],
                    op0=sub, op1=mult,
                )
                if ln_affine:  # on DVE: gpsimd is reserved for broadcasts
                    y_t = ypool.tile([128, D], f32, tag="y")
                    nc.vector.tensor_mul(y_t, xn, gamma_b)
                    nc.vector.tensor_add(y_t, y_t, beta_b)
                else:          # gamma==1, beta==0 (checked host-side)
                    y_t = xn
                nc.sync.dma_start(out=out_r[:, nt, :], in_=y_t)

            # ---- emission schedule ---------------------------------------
            # ramp: just enough projection work for pair 0 + first AV tiles
            q_proj(0, 0)
            q_proj(0, 1)
            for mc in range(MCH):
                k_proj(0, mc)
            v_proj(0)
            v_proj(1)

            def C(f, *a):
                return lambda: f(*a)

            # pair-0 fillers: V tiles JIT (AV of group g needs v(2g,2g+1);
            # slot g supplies v(2g+2,2g+3)); pair-p prereqs (qT/kT complete)
            # must be emitted before pair p starts
            f00 = {
                0: (C(v_proj, 2), C(v_proj, 3)),
                1: (C(v_proj, 4), C(v_proj, 5)),
                2: (C(v_proj, 6), C(v_proj, 7)),
                3: (C(v_proj, 8), C(v_proj, 9)),
                4: (C(v_proj, 10), C(v_proj, 11)),
                5: (C(v_proj, 12), C(v_proj, 13)),
                6: (C(v_proj, 14), C(v_proj, 15), C(q_proj, 1, 0)),
                7: (C(q_proj, 1, 1), C(k_proj, 1, 0)),
            }
            # k(t,mc) feeds score groups 2mc..2mc+1 of pair t: later chunks
            # can trail into pair t itself as long as they stay 2 groups ahead
            f10 = {
                0: (C(k_proj, 1, 1),),
                1: (C(k_proj, 1, 2), C(k_proj, 1, 3)),
                3: (C(q_proj, 2, 0),),
                4: (C(q_proj, 2, 1),),
                5: (C(k_proj, 2, 0),),
                6: (C(k_proj, 2, 1),),
                7: (C(k_proj, 2, 2), C(k_proj, 2, 3)),
            }
            f20 = {
                0: (C(q_proj, 3, 0),),
                1: (C(q_proj, 3, 1),),
                4: (C(k_proj, 3, 0),),
                5: (C(k_proj, 3, 1),),
                6: (C(k_proj, 3, 2), C(k_proj, 3, 3)),
            }
            # Scalar queue is strict FIFO: the wave-A Sqrt must enter it only
            # when its bn-stats deps are long done, else every later exp
            # stalls behind it.  fronts 0-3 early in ncc1, Sqrt a full pair
            # later, backs on the last pair.
            f01 = {2: (C(out_front, 0),), 4: (C(out_front, 1),),
                   6: (C(out_front, 2),)}
            f11 = {0: (C(out_front, 3),)}
            f21 = {4: (C(rstd_batch, (0, 1, 2, 3)),)}
            f31 = {0: (C(out_back, 0),), 2: (C(out_back, 1),),
                   4: (C(out_back, 2),), 6: (C(out_back, 3),)}

            pair_order = [(0, 0), (1, 0), (2, 0), (3, 0),
                          (0, 1), (1, 1), (2, 1), (3, 1)]
            fillmap = {(0, 0): f00, (1, 0): f10, (2, 0): f20,
                       (0, 1): f01, (1, 1): f11, (2, 1): f21, (3, 1): f31}
            attend_all(pair_order, fillmap)
            out_front(4)
            out_front(5)
            out_front(6)
            out_front(7)
            rstd_batch((4, 5, 6, 7))
            out_back(4, tail=True)
            out_back(5, tail=True)
            out_back(6, tail=True)
            out_back(7, tail=True)

    nc.compile()
    return nc


def kernel(**inputs):
    from concourse.bass_utils import run_bass_kernel_spmd

    gamma_a = np.asarray(inputs["gamma"], dtype=np.float32)
    beta_a = np.asarray(inputs["beta"], dtype=np.float32)
    ln_affine = bool(np.any(gamma_a != 1.0) or np.any(beta_a != 0.0))
    ck = ("nc", ln_affine)
    if ck not in _CACHE:
        _CACHE[ck] = _build(ln_affine)
    nc = _CACHE[ck]

    query = np.asarray(inputs["query"], dtype=np.float32)
    key = np.asarray(inputs["key"], dtype=np.float32)
    value = np.asarray(inputs["value"], dtype=np.float32)
    mask = np.asarray(inputs["mask"])
    WQ = np.asarray(inputs["WQ"], dtype=np.float32)
    WK = np.asarray(inputs["WK"], dtype=np.float32)
    WV = np.asarray(inputs["WV"], dtype=np.float32)
    WO = np.asarray(inputs["WO"], dtype=np.float32)
    bO = np.asarray(inputs["bO"], dtype=np.float32)
    gamma = np.asarray(inputs["gamma"], dtype=np.float32)
    beta = np.asarray(inputs["beta"], dtype=np.float32)

    scale = np.float32(1.0 / np.sqrt(HD))
    wqT = np.ascontiguousarray(WQ.T * scale).astype(BF16)
    wkT = np.ascontiguousarray(WK.T).astype(BF16)
    wvT = np.ascontiguousarray(WV.T).astype(BF16)
    woT = np.ascontiguousarray(WO.T).astype(BF16)
    gamma_in = gamma.reshape(1, D)
    beta_in = beta.reshape(1, D)
    mask_bin = (mask != 0)

    in_maps = []
    for c in range(NCORES):
        b, n0 = c // 2, (c % 2) * NS
        # mask, transposed and prepacked per (n-chunk, score-group):
        # maskP[ncc, g, p, u*512+nn] = maskT[g*256+u*128+p, ncc*512+nn]
        mT = np.ascontiguousarray(mask_bin[b, n0 : n0 + NS, :].T)  # [M, NS]
        mP = (
            mT.reshape(8, 2, 128, 2, 512)
            .transpose(3, 0, 2, 1, 4)
            .reshape(2 * 8 * 128, 1024)
        )
        in_maps.append({
            "xqT": np.ascontiguousarray(query[b, n0 : n0 + NS, :].T).astype(BF16),
            "xkT": np.ascontiguousarray(key[b].T).astype(BF16),
            "xvT": np.ascontiguousarray(value[b].T).astype(BF16),
            "maskP": np.ascontiguousarray(mP).astype(BF16),
            "qres": np.ascontiguousarray(query[b, n0 : n0 + NS, :] + bO[None, :]),
            "wqT": wqT, "wkT": wkT, "wvT": wvT, "woT": woT,
            "gamma": gamma_in, "beta": beta_in,
        })

    trace = bool(int(os.environ.get("BASS_KERNEL_TRACE", "0")))
    res = run_bass_kernel_spmd(nc, in_maps, core_ids=list(range(NCORES)), trace=trace)
    _CACHE["last_results"] = res

    out = np.empty((B, N, D), dtype=np.float32)
    for c in range(NCORES):
        b, n0 = c // 2, (c % 2) * NS
        out[b, n0 : n0 + NS, :] = res.results[c]["out"]
    return out

